# revision 7
# baseline (speedup 1.0000x reference)
# MoE (top-2 of 16 experts) Trainium2 kernel — expert parallelism over 8 cores.
#
# Strategy (per the expert-parallel sharding hint):
#   * Host computes the (tiny) router: softmax over E=16, top-2, renormalize —
#     this decides the sharding, i.e. which token rows are dispatched to which
#     core. Each of the 8 cores owns 2 experts.
#   * Tokens are dispatched (gathered + transposed) per expert on the host into
#     fixed-capacity buffers — this is the "all-to-all dispatch" step, done
#     while building the per-core SPMD input maps.
#   * Each core streams its 2 experts' weights (the dominant memory traffic,
#     ~35 MB/core) and computes  y_e^T = W_down @ (silu(W_gate x^T) * (W_up x^T))
#     for its gathered tokens with fp32 storage and fp32r matmuls.
#   * Host combines: y[n] = sum_k w_k(n) * y_{e_k(n)}[slot_k(n)]  (the
#     "all-to-all combine"), exploiting that each token hits exactly 2 experts.
#
# Shapes are hardcoded for the problem instance:
#   x: (2048, 1024) f32, gate_w: (16, 1024) f32,
#   w_gate_up: (16, 2816, 1024) f32, w_down: (16, 1024, 1408) f32, top_k = 2.

import numpy as np

N, H, E, F = 2048, 1024, 16, 1408
TOPK = 2
NCORES = 8
EPC = E // NCORES  # experts per core

_COMPILED = {}  # capacity -> (nc, names)


def _build_bass(C: int, reps: int = 1):
    """Build + compile the SPMD per-core program for token capacity C.

    reps > 1 repeats the whole computation (timing amplification only).
    """
    from contextlib import ExitStack

    import concourse.bass as bass
    import concourse.tile as tile
    from concourse import bacc, mybir

    KH = H // 128   # 8  k-tiles over H (gate/up contraction)
    KF = F // 128   # 11 k-tiles over F (down contraction)
    MF = F // 128   # 11 m-blocks over F (gate/up output)
    MH = H // 128   # 8  m-blocks over H (down output)
    f32 = mybir.dt.float32
    f32r = mybir.dt.float32r

    nc = bacc.Bacc("TRN2", target_bir_lowering=False, debug=False,
                   num_devices=NCORES)

    xt = nc.dram_tensor("xt", [EPC, H, C], f32r, kind="ExternalInput").ap()
    wgu = nc.dram_tensor("wgu", [EPC, H, 2 * F], f32r, kind="ExternalInput").ap()
    wd = nc.dram_tensor("wd", [EPC, F, H], f32r, kind="ExternalInput").ap()
    yt = nc.dram_tensor("yt", [EPC, H, C], f32, kind="ExternalOutput").ap()

    with tile.TileContext(nc) as tc, ExitStack() as ctx:
        wpool = ctx.enter_context(tc.tile_pool(name="w", bufs=2))
        xpool = ctx.enter_context(tc.tile_pool(name="x", bufs=2))
        sgpool = ctx.enter_context(tc.tile_pool(name="sg", bufs=2))
        hpool = ctx.enter_context(tc.tile_pool(name="h", bufs=1))
        ypool = ctx.enter_context(tc.tile_pool(name="y", bufs=2))
        sigpool = ctx.enter_context(tc.tile_pool(name="sig", bufs=3))
        pspool = ctx.enter_context(
            tc.tile_pool(name="ps", bufs=4, space=bass.MemorySpace.PSUM))

        for e in [ee for _ in range(reps) for ee in range(EPC)]:
            xe = xpool.tile([128, KH, C], f32r, tag="x")
            nc.sync.dma_start(
                xe[:], xt[e].rearrange("(j p) c -> p j c", p=128))

            # ---- gate projection + silu ----
            wg = wpool.tile([128, KH, F], f32r, tag="w")
            nc.sync.dma_start(
                wg[:], wgu[e, :, 0:F].rearrange("(j p) f -> p j f", p=128))
            sg = sgpool.tile([128, MF, C], f32, tag="sg")
            for m in range(MF):
                ps = pspool.tile([128, C], f32, tag="ps")
                for k in range(KH):
                    nc.tensor.matmul(
                        ps[:],
                        wg[:, k, m * 128:(m + 1) * 128],
                        xe[:, k, :],
                        start=(k == 0), stop=(k == KH - 1))
                sig = sigpool.tile([128, C], f32, tag="sig")
                nc.scalar.activation(
                    sig[:], ps[:], mybir.ActivationFunctionType.Sigmoid)
                nc.vector.tensor_mul(sg[:, m, :], ps[:], sig[:])

            # ---- up projection, h = silu(gate) * up ----
            wu = wpool.tile([128, KH, F], f32r, tag="w")
            nc.sync.dma_start(
                wu[:], wgu[e, :, F:2 * F].rearrange("(j p) f -> p j f", p=128))
            h = hpool.tile([128, MF, C], f32r, tag="h")
            for m in range(MF):
                ps = pspool.tile([128, C], f32, tag="ps")
                for k in range(KH):
                    nc.tensor.matmul(
                        ps[:],
                        wu[:, k, m * 128:(m + 1) * 128],
                        xe[:, k, :],
                        start=(k == 0), stop=(k == KH - 1))
                nc.vector.tensor_mul(h[:, m, :], ps[:], sg[:, m, :])

            # ---- down projection ----
            wdn = wpool.tile([128, KF, H], f32r, tag="w")
            nc.sync.dma_start(
                wdn[:], wd[e].rearrange("(j p) h -> p j h", p=128))
            yo = ypool.tile([128, MH, C], f32, tag="y")
            for m in range(MH):
                ps = pspool.tile([128, C], f32, tag="ps")
                for k in range(KF):
                    nc.tensor.matmul(
                        ps[:],
                        wdn[:, k, m * 128:(m + 1) * 128],
                        h[:, k, :],
                        start=(k == 0), stop=(k == KF - 1))
                nc.vector.tensor_copy(yo[:, m, :], ps[:])
            nc.gpsimd.dma_start(
                yt[e].rearrange("(j p) c -> p j c", p=128), yo[:])

    nc.compile()
    return nc


def _route(x: np.ndarray, gate_w: np.ndarray):
    """Replicate the reference router in numpy float32."""
    logits = x @ gate_w.T                                   # (N, E)
    m = logits.max(-1, keepdims=True)
    p = np.exp((logits - m).astype(np.float32))
    p = (p / p.sum(-1, keepdims=True)).astype(np.float32)
    topk_ids = np.argsort(-p, axis=-1, kind="stable")[:, :TOPK]  # (N, K)
    topk_w = np.take_along_axis(p, topk_ids, axis=-1)
    topk_w = (topk_w / topk_w.sum(-1, keepdims=True)).astype(np.float32)
    return topk_ids, topk_w


def kernel(x, gate_w, w_gate_up, w_down, top_k):
    from concourse.bass_utils import run_bass_kernel_spmd

    x = np.ascontiguousarray(np.asarray(x, dtype=np.float32))
    gate_w = np.asarray(gate_w, dtype=np.float32)
    w_gate_up = np.asarray(w_gate_up, dtype=np.float32)
    w_down = np.asarray(w_down, dtype=np.float32)
    assert int(top_k) == TOPK

    topk_ids, topk_w = _route(x, gate_w)

    # Dispatch: per-expert gathered token matrices (transposed), capacity C.
    counts = np.bincount(topk_ids.ravel(), minlength=E)
    C = max(256, int(np.ceil(counts.max() / 128)) * 128)
    xt_all = np.zeros((E, H, C), dtype=np.float32)
    lin = np.zeros((N, TOPK), dtype=np.int64)  # token,k -> e*C + slot
    idx_per_e = []
    for e in range(E):
        tok_e, k_e = np.nonzero(topk_ids == e)
        idx_per_e.append(tok_e)
        xt_all[e, :, :len(tok_e)] = x[tok_e].T
        lin[tok_e, k_e] = e * C + np.arange(len(tok_e))

    key = C
    if key not in _COMPILED:
        _COMPILED[key] = _build_bass(C)
    nc = _COMPILED[key]

    in_maps = []
    for c in range(NCORES):
        es = slice(c * EPC, (c + 1) * EPC)
        in_maps.append({
            "xt": np.ascontiguousarray(xt_all[es]),
            "wgu": np.ascontiguousarray(w_gate_up[es].transpose(0, 2, 1)),
            "wd": np.ascontiguousarray(w_down[es].transpose(0, 2, 1)),
        })

    res = run_bass_kernel_spmd(nc, in_maps, core_ids=list(range(NCORES)))

    yt_all = np.concatenate([r["yt"] for r in res.results], axis=0)  # (E,H,C)
    y_slots = np.ascontiguousarray(yt_all.transpose(0, 2, 1)).reshape(E * C, H)
    y = (topk_w[:, 0:1] * y_slots[lin[:, 0]]
         + topk_w[:, 1:2] * y_slots[lin[:, 1]])
    return y.astype(np.float32)


# revision 8
# speedup vs baseline: 2.7969x; 2.7969x over previous
# MoE (top-2 of 16 experts) Trainium2 kernel — expert parallelism over 8 cores.
#
# Strategy (per the expert-parallel sharding hint):
#   * Host computes the (tiny) router: softmax over E=16, top-2, renormalize —
#     this decides the sharding, i.e. which token rows are dispatched to which
#     core. Each of the 8 cores owns 2 experts.
#   * Tokens are dispatched (gathered + transposed) per expert on the host into
#     fixed-capacity buffers — the "all-to-all dispatch" step, done while
#     building the per-core SPMD input maps.
#   * Each core streams its 2 experts' weights (the dominant memory traffic,
#     ~35 MB/core, chunked for DMA/compute overlap) and computes
#     y_e^T = W_down @ (silu(W_gate x^T) * (W_up x^T)) for its gathered tokens.
#     Weights/activations stay fp32 in memory; matmuls run as float32r
#     (full-rate PE) accumulating fp32 in PSUM.
#   * Host combines: y[n] = sum_k w_k(n) * y_{e_k(n)}[slot_k(n)]  (the
#     "all-to-all combine"), exploiting that each token hits exactly 2 experts.
#
# Shapes are hardcoded for the problem instance:
#   x: (2048, 1024) f32, gate_w: (16, 1024) f32,
#   w_gate_up: (16, 2816, 1024) f32, w_down: (16, 1024, 1408) f32, top_k = 2.

import numpy as np

N, H, E, F = 2048, 1024, 16, 1408
TOPK = 2
NCORES = 8
EPC = E // NCORES  # experts per core

_COMPILED = {}  # (capacity, reps) -> compiled Bass module


def _build_bass(C: int, reps: int = 1):
    """Build + compile the SPMD per-core program for token capacity C.

    reps > 1 repeats the whole computation (timing amplification only).
    """
    from contextlib import ExitStack

    import concourse.bass as bass
    import concourse.tile as tile
    from concourse import bacc, mybir

    KH = H // 128   # 8  k-tiles over H (gate/up contraction)
    KF = F // 128   # 11 k-tiles over F (down contraction)
    MF = F // 128   # 11 m-blocks over F (gate/up output)
    MH = H // 128   # 8  m-blocks over H (down output)
    KC_GU = 2       # k-tiles per streamed gate/up weight chunk
    KC_DN = 3       # k-tiles per streamed down weight chunk
    f32 = mybir.dt.float32
    f32r = mybir.dt.float32r

    nc = bacc.Bacc("TRN2", target_bir_lowering=False, debug=False,
                   num_devices=NCORES)

    xt = nc.dram_tensor("xt", [EPC, H, C], f32r, kind="ExternalInput").ap()
    wgu = nc.dram_tensor("wgu", [EPC, H, 2 * F], f32r,
                         kind="ExternalInput").ap()
    wd = nc.dram_tensor("wd", [EPC, F, H], f32r, kind="ExternalInput").ap()
    yt = nc.dram_tensor("yt", [EPC, H, C], f32, kind="ExternalOutput").ap()

    with tile.TileContext(nc) as tc, ExitStack() as ctx:
        wpool = ctx.enter_context(tc.tile_pool(name="w", bufs=8))
        xpool = ctx.enter_context(tc.tile_pool(name="x", bufs=2))
        sgpool = ctx.enter_context(tc.tile_pool(name="sg", bufs=2))
        hpool = ctx.enter_context(tc.tile_pool(name="h", bufs=1))
        ypool = ctx.enter_context(tc.tile_pool(name="y", bufs=2))
        sigpool = ctx.enter_context(tc.tile_pool(name="sig", bufs=3))
        pspool = ctx.enter_context(
            tc.tile_pool(name="ps", bufs=6, space=bass.MemorySpace.PSUM))

        def stream_w(src2d, nk, width, kc):
            """DMA weight k-tiles in chunks of kc; returns (k0, k1, tile)."""
            parts = []
            for k0 in range(0, nk, kc):
                k1 = min(k0 + kc, nk)
                wt = wpool.tile([128, k1 - k0, width], f32r, tag="w")
                nc.sync.dma_start(
                    wt[:], src2d[k0 * 128:k1 * 128, :]
                    .rearrange("(j p) w -> p j w", p=128))
                parts.append((k0, k1, wt))
            return parts

        for e in [ee for _ in range(reps) for ee in range(EPC)]:
            xe = xpool.tile([128, KH, C], f32r, tag="x")
            nc.sync.dma_start(
                xe[:], xt[e].rearrange("(j p) c -> p j c", p=128))

            sg = sgpool.tile([128, MF, C], f32, tag="sg")   # silu(gate)^T
            h = hpool.tile([128, MF, C], f32r, tag="h")     # (silu(g)*up)^T
            yo = ypool.tile([128, MH, C], f32, tag="y")     # y^T staging

            # ---- gate then up projection (feature-major: out is [F, C]) ----
            for half in range(2):
                wparts = stream_w(wgu[e, :, half * F:(half + 1) * F],
                                  KH, F, KC_GU)
                for m in range(MF):
                    ps = pspool.tile([128, C], f32, tag="ps")
                    for (k0, k1, wt) in wparts:
                        for k in range(k0, k1):
                            nc.tensor.matmul(
                                ps[:],
                                wt[:, k - k0, m * 128:(m + 1) * 128],
                                xe[:, k, :],
                                start=(k == 0), stop=(k == KH - 1))
                    if half == 0:
                        sig = sigpool.tile([128, C], f32, tag="sig")
                        nc.scalar.activation(
                            sig[:], ps[:],
                            mybir.ActivationFunctionType.Sigmoid)
                        nc.vector.tensor_mul(sg[:, m, :], ps[:], sig[:])
                    else:
                        nc.vector.tensor_mul(h[:, m, :], ps[:], sg[:, m, :])

            # ---- down projection: y^T = W_down @ h ----
            wparts = stream_w(wd[e], KF, H, KC_DN)
            for m in range(MH):
                ps = pspool.tile([128, C], f32, tag="ps")
                for (k0, k1, wt) in wparts:
                    for k in range(k0, k1):
                        nc.tensor.matmul(
                            ps[:],
                            wt[:, k - k0, m * 128:(m + 1) * 128],
                            h[:, k, :],
                            start=(k == 0), stop=(k == KF - 1))
                nc.vector.tensor_copy(yo[:, m, :], ps[:])
            nc.gpsimd.dma_start(
                yt[e].rearrange("(j p) c -> p j c", p=128), yo[:])

    nc.compile()
    return nc


def _route(x: np.ndarray, gate_w: np.ndarray):
    """Replicate the reference router in numpy float32."""
    logits = x @ gate_w.T                                   # (N, E)
    m = logits.max(-1, keepdims=True)
    p = np.exp((logits - m).astype(np.float32))
    p = (p / p.sum(-1, keepdims=True)).astype(np.float32)
    topk_ids = np.argsort(-p, axis=-1, kind="stable")[:, :TOPK]  # (N, K)
    topk_w = np.take_along_axis(p, topk_ids, axis=-1)
    topk_w = (topk_w / topk_w.sum(-1, keepdims=True)).astype(np.float32)
    return topk_ids, topk_w


def kernel(x, gate_w, w_gate_up, w_down, top_k):
    from concourse.bass_utils import run_bass_kernel_spmd

    x = np.ascontiguousarray(np.asarray(x, dtype=np.float32))
    gate_w = np.asarray(gate_w, dtype=np.float32)
    w_gate_up = np.asarray(w_gate_up, dtype=np.float32)
    w_down = np.asarray(w_down, dtype=np.float32)
    assert int(top_k) == TOPK

    topk_ids, topk_w = _route(x, gate_w)

    # Dispatch: per-expert gathered token matrices (transposed), capacity C.
    counts = np.bincount(topk_ids.ravel(), minlength=E)
    C = max(256, int(np.ceil(counts.max() / 128)) * 128)
    xt_all = np.zeros((E, H, C), dtype=np.float32)
    lin = np.zeros((N, TOPK), dtype=np.int64)  # token,k -> e*C + slot
    for e in range(E):
        tok_e, k_e = np.nonzero(topk_ids == e)
        xt_all[e, :, :len(tok_e)] = x[tok_e].T
        lin[tok_e, k_e] = e * C + np.arange(len(tok_e))

    key = (C, 1)
    if key not in _COMPILED:
        _COMPILED[key] = _build_bass(C)
    nc = _COMPILED[key]

    in_maps = []
    for c in range(NCORES):
        es = slice(c * EPC, (c + 1) * EPC)
        in_maps.append({
            "xt": np.ascontiguousarray(xt_all[es]),
            "wgu": np.ascontiguousarray(w_gate_up[es].transpose(0, 2, 1)),
            "wd": np.ascontiguousarray(w_down[es].transpose(0, 2, 1)),
        })

    res = run_bass_kernel_spmd(nc, in_maps, core_ids=list(range(NCORES)))

    yt_all = np.concatenate([r["yt"] for r in res.results], axis=0)  # (E,H,C)
    y_slots = np.ascontiguousarray(yt_all.transpose(0, 2, 1)).reshape(E * C, H)
    y = (topk_w[:, 0:1] * y_slots[lin[:, 0]]
         + topk_w[:, 1:2] * y_slots[lin[:, 1]])
    return y.astype(np.float32)
